# revision 45
# baseline (speedup 1.0000x reference)
"""MLA (multi-latent-head attention) Trainium2 kernel.

Problem: B=2, T=2048, D_MODEL=2048, N_HEAD=16, D_C=512, D_HEAD=128, D_ROPE=32.

Sharding: 8 cores = 2 batches x 4 head-groups (4 heads per core).
Each core computes, for its batch b and heads [4g..4g+3]:
  c_kv^T = W_DKV^T x^T          [512, T]   (bf16, transposed orientation)
  c_q^T  = W_DQ^T  x^T          [512, T]
  kr^T   = rope(W_KR_g^T x^T)   [128, T]   (4 heads x 32 rope dims)
  qr^T   = rope(W_QR_g^T c_q^T) [128, T]
  kc^T_h = W_UK_h^T c_kv^T      [128, T] per head
  qc^T_h = W_UQ_h^T c_q^T       [128, T] per head
  V      = c_kv W_UV_g          [T, 512]  (natural orientation, 4 heads)
  S^T    = K Q^T  (per k-tile of 128, accumulated over d=128 content + d=32 rope)
  P^T    = exp(S^T / sqrt(160))           (no max subtraction; |S|<~2 by construction)
  OUT^T  = V^T P^T  (PSUM accum over k-tiles)
  l      = softmax denom: bf16 running sum of P pair-tiles on DVE (2x 16-bit
           rate), one 512-col ones-matmul for the cross-partition reduce
           (with free row broadcast), then 1/l = exp(-ln(l)) on ACT --
           DVE reciprocal is ~6.5 ns/elem and would cost 54us.
  OUT^T normalized by broadcast(1/l) and written [512, T] fp32; host transposes.

RoPE: weight columns pre-permuted on host to [even dims(16) | odd dims(16)] per
head; rope computed as pre*cos + swap(pre)*sin_signed where swap() is a
permutation matmul (16-row block swap within each 32-row head block) and
sin_signed carries the sign flip for the first half.

Scheduling notes:
- Projection loops run k-tile-outer with 4 PSUM chunk accumulators so the PE
  starts as soon as the first x tile lands and each weight tile is loaded once.
- Attention processes k-tiles in pairs with a [128, 1024] S/P tile: one
  ACTIVATE per pair (halves the ACT 352-cycle overheads) and fewer PSUM
  switches.
- Engine budget (measured): the kernel is PE-streaming-bound (~92% busy).
  The former per-k-tile ones-matmul softmax denominator cost 131k PE cycles
  (55us); it now rides DVE (bf16 pair-tile sums) + one tiny ones-matmul per
  (head, q-chunk). Projection-phase PSUM evictions ride ACT (idle during
  phases 1-2); ACT's exp stream is the attention-phase near-bottleneck, so
  the 1/l path uses its Ln/Exp tables (same act table as exp -> no reloads).
- PSUM pools are phase-stacked: phases 1-2 use one 6-buf accumulator pool;
  phase 3 uses S-pair(4 banks)/OUT(2)/l(2).
- SBUF is phased too: x^T tiles + streamed W_DKV/W_DQ blocks live only during
  phase 1; per-head K/Q/V reuse that space (stack-ordered tile pools).
- Measured dead ends (this HW/toolchain): fp8 e4m3 for Q/K/P/V fails the
  2e-2 gate (1.9-3.9e-2 each -- softmax output has ~8x cancellation vs |v|);
  fp8 DoubleRow matmuls show NO column-rate win on HW (~420ns vs ~380ns for
  bf16 512-col); gpsimd tensor_tensor(divide) and partition_all_reduce are
  rejected by this walrus build (engine check / ISA length).
"""

import sys

if "/opt/trn_rl_repo" not in sys.path:
    sys.path.insert(0, "/opt/trn_rl_repo")

import math

import ml_dtypes
import numpy as np

import concourse.bass as bass
import concourse.bass_isa as bass_isa
import concourse.mybir as mybir
import concourse.tile as tile
from concourse.vector_clock import ScopedClock
from concourse.bass_utils import run_bass_kernel_spmd

BF16 = ml_dtypes.bfloat16

B, T, DM, NH, DC = 2, 2048, 2048, 16, 512
DH = DM // NH            # 128
DR = DC // NH            # 32
HL = 4                   # heads per core
D_ATT = DH + DR          # 160
SCALE = 1.0 / math.sqrt(D_ATT)

NKT_DM = DM // 128       # 16 k-tiles over d_model
NKT_DC = DC // 128       # 4  k-tiles over d_c
NTT = T // 128           # 16 tiles over T (k-tiles of attention)
NCH = T // 512           # 4  chunks of 512 over T
F32 = mybir.dt.float32
BF = mybir.dt.bfloat16
FP8 = mybir.dt.float8e4

# fp8 DoubleRow for the 32-dim rope half of the score matmul. Measured on
# HW: DR matmuls run at ~420ns vs ~380ns for plain bf16 512-col matmuls
# (no 0.5 cyc/col win materializes; LDWEIGHTS loses FWL), and rope-fp8
# costs 1.0e-2 rel err. Strictly worse -> keep disabled.
ROPE_FP8 = False

# Tensor-parallel c_q: the 4 cores of a batch each compute one 128-row
# latent quarter of c_q^T and AllGather the quarters (saves 98k PE
# cycles, ~41us/core). Measured: the gather DOES hide behind kc/V
# (only 7us of PE stalls; the ~75us collective-bootstrap is anchored
# to NEFF entry so only c_q's late consumer works), BUT any NEFF
# containing collectives runs the PE at ~2.08 GHz instead of 2.4
# (uniform 437ns vs 379ns matmuls, the power-profile downclock) -- a
# flat ~15% tax (~44us) that cancels the savings. Net +17us -> OFF.
TP_CQ = False
NJ_DOWN = 6 if TP_CQ else 9


class PatchedTC(tile.TileContext):
    """This walrus build rejects >1 sync-wait on CTRL (Drain) instructions;
    split the final tile drain into one drain per semaphore wait."""

    def _drain_and_barrier(self, tick_clock, wait_clock):
        drain_inst = self.nc.sync.drain()
        wait_clock.add_sem_waits(
            drain_inst.ins, ScopedClock({None: tick_clock.global_clock})
        )
        si = drain_inst.ins.sync_info
        if si is not None and si.on_wait and len(si.on_wait) > 1:
            waits = list(si.on_wait)
            si.on_wait = waits[:1]
            for w in waits[1:]:
                d2 = self.nc.sync.drain()
                d2.ins.sync_info = mybir.SyncInfo(on_wait=[w], on_update=[])
        self.nc.all_engine_barrier()
        assert self.sems is not None
        popped = self.nc._tile_sem_poison_stack.pop()
        assert popped is self._sem_poison
        self.nc.clear_and_free_semaphores(list(self.sems.allocated().values()))
        self.nc.all_engine_barrier()


def _split_multi_waits(nc):
    """This walrus build rejects >1 sync-wait per instruction: move extra
    waits onto NoOp instructions inserted before the owner on its engine."""
    n = 0
    for fn in nc.m.functions:
        for bb in fn.blocks:
            out = []
            changed = False
            for inst in bb.instructions:
                si = inst.sync_info
                if si is not None and si.on_wait and len(si.on_wait) > 1:
                    waits = list(si.on_wait)
                    for w in waits[:-1]:
                        n += 1
                        nop = mybir.InstNoOp(
                            name=f"{inst.name}_w{n}", ins=[], outs=[],
                            sync_info=mybir.SyncInfo(on_wait=[w], on_update=[]),
                        )
                        nop.engine = inst.engine
                        out.append(nop)
                    si.on_wait = waits[-1:]
                    inst.sync_info = si
                    changed = True
                out.append(inst)
            if changed:
                bb.instructions = out


def _build_nc(with_biases):
    """Build the SPMD Bass program (identical on all cores; data differs)."""
    nc = bass.Bass()

    # ---- HBM inputs (per-core shards; layouts produced by host prep) ----
    xt = nc.dram_tensor("xt", [NKT_DM, 128, T], BF, kind="ExternalInput")
    # wdown column-blocks of [128, 16*128]:
    #   TP_CQ: j=0 own W_DQ quarter, j=1..4 W_DKV, j=5 W_KR
    #   else:  j=0..3 W_DKV, 4..7 W_DQ, 8 W_KR
    wdown = nc.dram_tensor("wdown", [NJ_DOWN, 128, NKT_DM * 128], BF,
                           kind="ExternalInput")
    wuk = nc.dram_tensor("wuk", [128, NKT_DC * 512], BF, kind="ExternalInput")
    wuv = nc.dram_tensor("wuv", [128, NKT_DC * 512], BF, kind="ExternalInput")
    wuq = nc.dram_tensor("wuq", [128, NKT_DC * 512], BF, kind="ExternalInput")
    wqr = nc.dram_tensor("wqr", [128, NKT_DC * 128], BF, kind="ExternalInput")
    cos_d = nc.dram_tensor("cos", [128, T], F32, kind="ExternalInput")
    sin_d = nc.dram_tensor("sin", [128, T], F32, kind="ExternalInput")
    swp_d = nc.dram_tensor("swp", [128, 128], BF, kind="ExternalInput")
    ones_d = nc.dram_tensor("ones128", [128, 512], BF, kind="ExternalInput")
    if with_biases:
        # cols follow the wdown block layout above
        bias1_d = nc.dram_tensor("bias1", [128, NJ_DOWN], F32,
                                 kind="ExternalInput")
        # [128, 9]: cols 0-3 b_UK(g), 4-7 b_UQ(g), 8 b_QR(g, permuted)
        bias2_d = nc.dram_tensor("bias2", [128, 9], F32, kind="ExternalInput")
        biasv_d = nc.dram_tensor("biasv", [128, 512], F32, kind="ExternalInput")

    out_t = nc.dram_tensor("out_t", [HL * 128, T], F32, kind="ExternalOutput")

    with PatchedTC(nc) as tc:
        # ---- persistent pools (bottom of the SBUF stack) ----
        with tc.tile_pool(name="consts", bufs=1) as consts, \
             tc.tile_pool(name="cpool", bufs=1) as cpool, \
             tc.tile_pool(name="ppool", bufs=4) as ppool, \
             tc.tile_pool(name="small", bufs=3) as small, \
             tc.tile_pool(name="opool", bufs=3) as opool, \
             tc.tile_pool(name="dramp", bufs=1, space="DRAM") as dramp:
            ib_cq = ob_cq = None
            if TP_CQ:
                # DRAM bounce buffers: collectives cannot target SBUF or
                # kernel I/O tensors directly
                ib_cq = dramp.tile([128, T], BF)
                ob_cq = dramp.tile([4 * 128, T], BF)

            # ---- constants (tiles now; DMAs deferred until after the
            # x tiles + first weight block are queued, so phase 1 starts
            # as early as possible) ----
            w_uk = consts.tile([128, NKT_DC * 512], BF, tag="wuk")
            w_uv = consts.tile([128, NKT_DC * 512], BF, tag="wuv")
            w_uq = consts.tile([128, NKT_DC * 512], BF, tag="wuq")
            w_qr = consts.tile([128, NKT_DC * 128], BF, tag="wqr")
            cos_t = consts.tile([128, T], F32, tag="cos")
            sin_t = consts.tile([128, T], F32, tag="sin")
            swp_t = consts.tile([128, 128], BF, tag="swp")
            ones128 = consts.tile([128, 512], BF, tag="ones128")
            bias1 = bias2 = biasv = None
            if with_biases:
                bias1 = consts.tile([128, NJ_DOWN], F32, tag="bias1")
                bias2 = consts.tile([128, 9], F32, tag="bias2")
                biasv = consts.tile([128, 512], F32, tag="biasv")

            def dma_consts():
                nc.sync.dma_start(out=cos_t, in_=cos_d[:])
                nc.sync.dma_start(out=sin_t, in_=sin_d[:])
                nc.sync.dma_start(out=w_uk, in_=wuk[:])
                nc.sync.dma_start(out=w_uv, in_=wuv[:])
                nc.sync.dma_start(out=w_uq, in_=wuq[:])
                nc.sync.dma_start(out=w_qr, in_=wqr[:])
                if with_biases:
                    nc.sync.dma_start(out=bias1, in_=bias1_d[:])
                    nc.sync.dma_start(out=bias2, in_=bias2_d[:])
                    nc.sync.dma_start(out=biasv, in_=biasv_d[:])

            # persistent phase-1 outputs
            ckv_t = [cpool.tile([128, T], BF, tag=f"ckv{j}", name=f"ckv{j}")
                     for j in range(4)]
            cq_t = [cpool.tile([128, T], BF, tag=f"cq{j}", name=f"cq{j}")
                    for j in range(4)]
            kr_pre = cpool.tile([128, T], BF, tag="kr_pre")
            kr_t = cpool.tile([128, T], FP8 if ROPE_FP8 else BF, tag="kr")
            cq_own = None
            if TP_CQ:
                cq_own = cpool.tile([128, T], BF, tag="cq_own")
            # packed rope operands for the DoubleRow score matmul:
            # head h lives on partitions 32h..32h+16, rope dim 16j+p in
            # slot j (the same (p,j) pairing on both K and Q sides)
            krp = qrp = None
            if ROPE_FP8:
                krp = cpool.tile([128, 2, T], FP8, tag="krp")
            # warm the ACT exp table (one-time ~2.7us load) off the
            # critical path, before attention needs it
            warm = small.tile([1, 1], F32, tag="warm")
            nc.vector.memset(warm, 0.0)
            nc.scalar.activation(out=warm, in_=warm,
                                 func=mybir.ActivationFunctionType.Exp)


            def evict(dst_ap, src_psum, bias_ap):
                if bias_ap is not None:
                    nc.vector.tensor_scalar_add(out=dst_ap, in0=src_psum,
                                                scalar1=bias_ap)
                    return
                # projection-phase evictions ride the otherwise-idle ACT
                # engine; DVE is reserved for rope + the attention-phase
                # softmax-denominator accumulation
                nc.scalar.activation(
                    out=dst_ap, in_=src_psum,
                    func=mybir.ActivationFunctionType.Copy)

            # ===== phases 1-2: k-outer projections, 4 chunk accumulators ====
            # (PSUM pool closed before phase 3; PSUM/SBUF pool stacks are
            # independent, so this interleaves fine with the SBUF pools.)
            prps_cm = tc.tile_pool(name="prps", bufs=6, space="PSUM")
            prps = prps_cm.__enter__()
            if True:

                def proj(lhs_tiles, src_tiles, dst, b_ap, tag):
                    nkt = len(lhs_tiles)
                    pss = [prps.tile([128, 512], F32, tag="prps",
                                     name=f"{tag}ps{ch}") for ch in range(NCH)]
                    for kt in range(nkt):
                        for ch in range(NCH):
                            nc.tensor.matmul(
                                pss[ch],
                                lhsT=lhs_tiles[kt],
                                rhs=src_tiles[kt][:, ch * 512: (ch + 1) * 512],
                                start=(kt == 0),
                                stop=(kt == nkt - 1),
                            )
                    for ch in range(NCH):
                        evict(dst[:, ch * 512: (ch + 1) * 512], pss[ch], b_ap)

                def apply_rope(pre_tile, dsts):
                    """out = pre*cos + swap(pre)*sin_signed.
                    dsts: [(tile, row_slice)] destinations for the final add."""
                    for ch in range(NCH):
                        sl = slice(ch * 512, (ch + 1) * 512)
                        sw = prps.tile([128, 512], F32, tag="prps",
                                       name="swpsum")
                        nc.tensor.matmul(sw, lhsT=swp_t[:], rhs=pre_tile[:, sl],
                                         start=True, stop=True)
                        t1 = opool.tile([128, 512], F32, tag="rope_t1")
                        nc.vector.tensor_mul(out=t1, in0=pre_tile[:, sl],
                                             in1=cos_t[:, sl])
                        t2 = opool.tile([128, 512], F32, tag="rope_t2")
                        nc.vector.tensor_mul(out=t2, in0=sw, in1=sin_t[:, sl])
                        for dst, rp in dsts:
                            nc.vector.tensor_add(out=dst[rp, sl],
                                                 in0=t1[rp, :], in1=t2[rp, :])

                # ---- PHASE 1: x^T consumers (x + streamed W resident) ----
                with tc.tile_pool(name="xpool", bufs=1) as xpool, \
                     tc.tile_pool(name="wstream", bufs=2) as wstream:
                    # HAM warm-up: swp+ones land in <1us; ~20 bf16 512-col
                    # matmuls (~8.5us at the cold 1.2 GHz clock) keep the
                    # PE busy through the ~12us x/wt DMA wait so phase 1
                    # starts at the warm 2.4 GHz clock. Sized to end just
                    # before x lands: results are never read.
                    nc.sync.dma_start(out=swp_t, in_=swp_d[:])
                    nc.sync.dma_start(out=ones128, in_=ones_d[:])
                    warm_ps = prps.tile([128, 512], F32, tag="prps",
                                        name="warmps")
                    for _ in range(20):
                        nc.tensor.matmul(warm_ps, lhsT=swp_t[:],
                                         rhs=ones128[:],
                                         start=True, stop=True)
                    wt0 = wstream.tile([128, NKT_DM * 128], BF, tag="wt",
                                       name="wt0")
                    nc.scalar.dma_start(out=wt0, in_=wdown[0])
                    x_tiles = []
                    for kt in range(NKT_DM):
                        xtile = xpool.tile([128, T], BF, tag=f"x{kt}",
                                           name=f"x{kt}")
                        nc.sync.dma_start(out=xtile, in_=xt[kt])
                        x_tiles.append(xtile)
                    dma_consts()

                    for j in range(NJ_DOWN):
                        if j == 0:
                            wt = wt0
                        else:
                            wt = wstream.tile([128, NKT_DM * 128], BF,
                                              tag="wt", name=f"wt{j}")
                            nc.scalar.dma_start(out=wt, in_=wdown[j])
                        if TP_CQ:
                            if j == 0:
                                dst = cq_own
                            elif j < 5:
                                dst = ckv_t[j - 1]
                            else:
                                dst = kr_pre
                            b_ap = bias1[:, j:j + 1] if with_biases else None
                        elif j < 4:
                            dst = ckv_t[j]
                            b_ap = bias1[:, j:j + 1] if with_biases else None
                        elif j < 8:
                            dst = cq_t[j - 4]
                            b_ap = bias1[:, j:j + 1] if with_biases else None
                        else:
                            dst = kr_pre
                            b_ap = bias1[:, 8:9] if with_biases else None
                        proj([wt[:, kt * 128: (kt + 1) * 128]
                              for kt in range(NKT_DM)],
                             x_tiles, dst, b_ap, f"p1j{j}")
                        # c_q quarter is block 0: launch its AllGather
                        # immediately so the ~75us collective bootstrap +
                        # wire hide under c_kv/kr matmuls and kc/V
                        if TP_CQ and j == 0:
                            nc.gpsimd.dma_start(ib_cq[:], cq_own[:])
                            nc.gpsimd.collective_compute(
                                "AllGather", mybir.AluOpType.bypass,
                                replica_groups=[[0, 1, 2, 3], [4, 5, 6, 7]],
                                ins=[ib_cq.opt()], outs=[ob_cq.opt()])
                            for jq in range(4):
                                nc.sync.dma_start(
                                    out=cq_t[jq],
                                    in_=ob_cq[128 * jq: 128 * (jq + 1), :])

                apply_rope(kr_pre, [(kr_t, slice(0, 128))])
                if ROPE_FP8:
                    for h in range(HL):
                        for j in range(2):
                            nc.sync.dma_start(
                                out=krp[32 * h: 32 * h + 16, j, :],
                                in_=kr_t[32 * h + 16 * j:
                                         32 * h + 16 * j + 16, :])

                # ---- PHASE 2+3: latent consumers (reuse x's SBUF) ----
                with tc.tile_pool(name="kqpool", bufs=1) as kqpool, \
                     tc.tile_pool(name="vpool", bufs=1) as vpool:
                    kc_t = [kqpool.tile([128, T], BF, tag=f"kc{h}",
                                        name=f"kc{h}") for h in range(HL)]
                    qc_t = [kqpool.tile([128, T], BF, tag=f"qc{h}",
                                        name=f"qc{h}") for h in range(HL)]
                    qr_pre = kqpool.tile([128, T], BF, tag="qr_pre")
                    qr_rope = kqpool.tile([128, T], FP8 if ROPE_FP8 else BF,
                                          tag="qr_rope")
                    if ROPE_FP8:
                        qrp = kqpool.tile([128, 2, T], FP8, tag="qrpk")
                    else:
                        # per-head zero-padded rope Q: only rows 32h..32h+32
                        # live, so the rope S matmul is a plain (0,0) matmul
                        qr_pad = [kqpool.tile([128, T], BF, tag=f"qrp{h}",
                                              name=f"qrp{h}")
                                  for h in range(HL)]
                        for h in range(HL):
                            nc.gpsimd.memset(qr_pad[h][:], 0.0)

                    for h in range(HL):
                        proj([w_uk[:, kt * 512 + 128 * h:
                                   kt * 512 + 128 * (h + 1)]
                              for kt in range(NKT_DC)],
                             ckv_t, kc_t[h],
                             bias2[:, h:h + 1] if with_biases else None,
                             f"p2k{h}")
                    for h in range(HL):
                        proj([w_uq[:, kt * 512 + 128 * h:
                                   kt * 512 + 128 * (h + 1)]
                              for kt in range(NKT_DC)],
                             cq_t, qc_t[h],
                             bias2[:, 4 + h:5 + h] if with_biases else None,
                             f"p2q{h}")
                    proj([w_qr[:, kt * 128: (kt + 1) * 128]
                          for kt in range(NKT_DC)],
                         cq_t, qr_pre,
                         bias2[:, 8:9] if with_biases else None, "p2r")

                    # rope once full-width (1 DVE add per chunk instead
                    # of 4 quarter-height ones), then repack per head on
                    # the DMA engines; the V matmuls BELOW then cover the
                    # rope DVE chain + scatter so attention starts with
                    # its Q inputs already resident (moving V earlier
                    # measured +14us)
                    apply_rope(qr_pre, [(qr_rope, slice(0, 128))])
                    if ROPE_FP8:
                        for h in range(HL):
                            for j in range(2):
                                nc.sync.dma_start(
                                    out=qrp[32 * h: 32 * h + 16, j, :],
                                    in_=qr_rope[32 * h + 16 * j:
                                                32 * h + 16 * j + 16, :])
                    else:
                        for h in range(HL):
                            rs = slice(32 * h, 32 * h + 32)
                            nc.sync.dma_start(out=qr_pad[h][rs, :],
                                              in_=qr_rope[rs, :])

                    # V natural: [T-tile rows, 512 (4 heads x 128)]
                    v_nat = []
                    for tt in range(NTT):
                        ps = prps.tile([128, 512], F32, tag="prps",
                                       name=f"vps{tt}")
                        for kt in range(NKT_DC):
                            nc.tensor.matmul(
                                ps,
                                lhsT=ckv_t[kt][:, tt * 128: (tt + 1) * 128],
                                rhs=w_uv[:, kt * 512: (kt + 1) * 512],
                                start=(kt == 0),
                                stop=(kt == NKT_DC - 1),
                            )
                        vt = vpool.tile([128, 512], BF, tag=f"v{tt}",
                                        name=f"v{tt}")
                        if with_biases:
                            nc.vector.tensor_add(out=vt, in0=ps, in1=biasv)
                        else:
                            nc.vector.tensor_copy(out=vt, in_=ps)
                        v_nat.append(vt)

                    # ========== PHASE 3: attention ==========
                    prps_cm.__exit__(None, None, None)
                    with tc.tile_pool(name="sp2", bufs=2, space="PSUM") as sp2, \
                         tc.tile_pool(name="opp", bufs=2,
                                      space="PSUM") as op_pool, \
                         tc.tile_pool(name="lpp", bufs=2,
                                      space="PSUM") as lp_pool, \
                         tc.tile_pool(name="lpool", bufs=2) as lpool:
                        for h in range(HL):
                            for qc in range(NCH):
                                qsl = slice(qc * 512, (qc + 1) * 512)
                                outp = op_pool.tile([128, 512], F32,
                                                    tag="outp",
                                                    name=f"outp{h}_{qc}")
                                # softmax denominator: bf16 running sum of
                                # the P pair-tiles on DVE (2x 16-bit rate),
                                # then a gpsimd cross-partition reduce; the
                                # PE does no l work at all.
                                acc = lpool.tile([128, 1024], BF, tag="acc",
                                                 name=f"acc{h}_{qc}")
                                # software pipeline: PV of pair kp runs
                                # after S of pair kp+1, so the PE never waits
                                # on the ACT exp.
                                pts = [None] * (NTT // 2)

                                def pv(kp):
                                    for ki in range(2):
                                        kt = 2 * kp + ki
                                        psl = slice(512 * ki, 512 * (ki + 1))
                                        nc.tensor.matmul(
                                            outp,
                                            lhsT=v_nat[kt][:, 128 * h:
                                                           128 * (h + 1)],
                                            rhs=pts[kp][:, psl],
                                            start=(kt == 0),
                                            stop=(kt == NTT - 1))

                                for kp in range(NTT // 2):
                                    spt = sp2.tile([128, 1024], F32, tag="sp",
                                                   name=f"sp{h}_{qc}_{kp}")
                                    for ki in range(2):
                                        kt = 2 * kp + ki
                                        ksl = slice(kt * 128, (kt + 1) * 128)
                                        half = spt[:, 512 * ki: 512 * (ki + 1)]
                                        nc.tensor.matmul(
                                            half, lhsT=kc_t[h][:, ksl],
                                            rhs=qc_t[h][:, qsl],
                                            start=True, stop=False)
                                        if ROPE_FP8:
                                            hp = slice(32 * h, 32 * h + 16)
                                            nc.tensor.matmul(
                                                half, lhsT=krp[hp, :, ksl],
                                                rhs=qrp[hp, :, qsl],
                                                start=False, stop=True,
                                                perf_mode=mybir.
                                                MatmulPerfMode.DoubleRow,
                                                tile_position=(32 * h, 0))
                                        else:
                                            nc.tensor.matmul(
                                                half, lhsT=kr_t[:, ksl],
                                                rhs=qr_pad[h][:, qsl],
                                                start=False, stop=True)
                                    pt = ppool.tile([128, 1024], BF, tag="pt")
                                    nc.scalar.activation(
                                        out=pt, in_=spt,
                                        func=mybir.ActivationFunctionType.Exp,
                                        scale=SCALE)
                                    pts[kp] = pt
                                    if kp == 1:
                                        nc.vector.tensor_add(
                                            out=acc, in0=pts[0], in1=pts[1])
                                    elif kp > 1:
                                        nc.vector.tensor_add(
                                            out=acc, in0=acc, in1=pt)
                                    if kp > 0:
                                        pv(kp - 1)
                                pv(NTT // 2 - 1)
                                accf = lpool.tile([128, 512], BF, tag="accf",
                                                  name=f"accf{h}_{qc}")
                                nc.vector.tensor_add(out=accf,
                                                     in0=acc[:, 0:512],
                                                     in1=acc[:, 512:1024])
                                # partition-reduce + row broadcast of l in
                                # one cheap 512-col ones-matmul
                                lacc = lp_pool.tile([128, 512], F32,
                                                    tag="lacc",
                                                    name=f"lacc{h}_{qc}")
                                nc.tensor.matmul(lacc, lhsT=ones128[:, 0:128],
                                                 rhs=accf, start=True,
                                                 stop=True)
                                # 1/l = exp(-ln(l)) on ACT (DVE reciprocal
                                # is ~6.5ns/elem; ACT table ops are ~0.9ns
                                # and Ln/Exp/Copy share one act table)
                                lnl = lpool.tile([128, 512], F32, tag="lnl",
                                                 name=f"lnl{h}_{qc}")
                                nc.scalar.activation(
                                    out=lnl, in_=lacc,
                                    func=mybir.ActivationFunctionType.Ln)
                                rinv = lpool.tile([128, 512], F32, tag="rinv",
                                                  name=f"rinv{h}_{qc}")
                                nc.scalar.activation(
                                    out=rinv, in_=lnl, scale=-1.0,
                                    func=mybir.ActivationFunctionType.Exp)
                                o_sb = opool.tile([128, 512], F32, tag="o_sb")
                                nc.vector.tensor_mul(out=o_sb, in0=outp,
                                                     in1=rinv)
                                nc.sync.dma_start(
                                    out=out_t[128 * h: 128 * (h + 1), qsl],
                                    in_=o_sb)

    _split_multi_waits(nc)
    return nc


_nc_cache = {}


def _get_nc(with_biases):
    if with_biases not in _nc_cache:
        _nc_cache[with_biases] = _build_nc(with_biases)
    return _nc_cache[with_biases]


def _rope_perm():
    """Permutation of the 32 rope dims within one head: evens then odds."""
    return np.concatenate([np.arange(0, DR, 2), np.arange(1, DR, 2)])


def kernel(x, W_DKV, b_DKV, W_UK, b_UK, W_UV, b_UV, W_DQ, b_DQ,
           W_UQ, b_UQ, W_QR, b_QR, W_KR, b_KR):
    x = np.asarray(x, np.float32)
    f32 = lambda a: np.asarray(a, np.float32)
    W_DKV, W_UK, W_UV, W_DQ, W_UQ, W_QR, W_KR = map(
        f32, (W_DKV, W_UK, W_UV, W_DQ, W_UQ, W_QR, W_KR))
    b_DKV, b_UK, b_UV, b_DQ, b_UQ, b_QR, b_KR = map(
        f32, (b_DKV, b_UK, b_UV, b_DQ, b_UQ, b_QR, b_KR))

    with_biases = any(np.any(b)
                      for b in (b_DKV, b_UK, b_UV, b_DQ, b_UQ, b_QR, b_KR))
    nc = _get_nc(with_biases)

    perm = _rope_perm()

    # lhsT-tile layout helper: W [K, C] -> [128, (K//128)*C], [p, kt*C + c]
    def tile_k(w):
        k, c = w.shape
        return np.ascontiguousarray(
            w.reshape(k // 128, 128, c).transpose(1, 0, 2).reshape(128, -1)
        ).astype(BF16)

    # column-block layout for streamed down-proj weights:
    # W [2048, C] -> per 128-col block j: [128, 16*128], [p, kt*128 + cc]
    def tile_k_blocks(w):
        k, c = w.shape
        nj = c // 128
        return np.ascontiguousarray(
            w.reshape(k // 128, 128, nj, 128).transpose(2, 1, 0, 3)
            .reshape(nj, 128, -1)
        ).astype(BF16)

    # x^T per batch, tiled over d_model: [16, 128, T]
    xt_b = []
    for b in range(B):
        xT = np.ascontiguousarray(x[b].T.astype(BF16))       # [DM, T]
        xt_b.append(np.ascontiguousarray(xT.reshape(NKT_DM, 128, T)))

    # RoPE tables: [128, T] fp32; rows 32h+i / 32h+16+i use freq i
    freqs = 10000.0 ** (-(np.arange(0, DR, 2, dtype=np.float64) / DR))   # [16]
    theta = np.arange(T, dtype=np.float64)[:, None] * freqs[None, :]     # [T, 16]
    cos16 = np.cos(theta).T.astype(np.float32)                           # [16, T]
    sin16 = np.sin(theta).T.astype(np.float32)
    cos_full = np.tile(cos16, (8, 1))                                    # [128, T]
    sin_signed = np.tile(np.concatenate([-sin16, sin16], 0), (4, 1))     # [128, T]

    # swap permutation matrix (16-row block swap inside each 32-row block)
    swp = np.zeros((128, 128), np.float32)
    for hb in range(4):
        for i in range(16):
            swp[32 * hb + 16 + i, 32 * hb + i] = 1.0
            swp[32 * hb + i, 32 * hb + 16 + i] = 1.0
    swp = swp.astype(BF16)

    ones128 = np.ones((128, 512), BF16)

    in_maps = []
    for c in range(8):
        b, g = divmod(c, 4)
        heads = slice(4 * g * DH, (4 * g + HL) * DH)          # content cols
        rcols = np.concatenate(
            [(4 * g + h) * DR + perm for h in range(HL)])     # rope cols
        if TP_CQ:
            # this core computes latent quarter g of c_q (block 0, first
            # so its AllGather fires early); c_kv stays replicated
            wdown = np.concatenate([
                tile_k_blocks(W_DQ)[g:g + 1],    # j=0: own c_q quarter
                tile_k_blocks(W_DKV),            # j=1..4
                tile_k_blocks(W_KR[:, rcols]),   # j=5
            ], axis=0)
        else:
            wdown = np.concatenate([
                tile_k_blocks(W_DKV),            # j=0..3
                tile_k_blocks(W_DQ),             # j=4..7
                tile_k_blocks(W_KR[:, rcols]),   # j=8
            ], axis=0)
        m = {
            "xt": xt_b[b],
            "wdown": wdown,
            "wuk": tile_k(W_UK[:, heads]),
            "wuv": tile_k(W_UV[:, heads]),
            "wuq": tile_k(W_UQ[:, heads]),
            "wqr": tile_k(W_QR[:, rcols]),
            "cos": cos_full,
            "sin": sin_signed,
            "swp": swp,
            "ones128": ones128,
        }
        if with_biases:
            bias1 = np.zeros((128, NJ_DOWN), np.float32)
            if TP_CQ:
                bias1[:, 0] = b_DQ[128 * g: 128 * (g + 1)]
                bias1[:, 1:5] = b_DKV.reshape(4, 128).T
                bias1[:, 5] = b_KR[rcols]
            else:
                bias1[:, 0:4] = b_DKV.reshape(4, 128).T
                bias1[:, 4:8] = b_DQ.reshape(4, 128).T
                bias1[:, 8] = b_KR[rcols]
            bias2 = np.zeros((128, 9), np.float32)
            bias2[:, 0:4] = b_UK[heads].reshape(4, 128).T
            bias2[:, 4:8] = b_UQ[heads].reshape(4, 128).T
            bias2[:, 8] = b_QR[rcols]
            m["bias1"] = bias1
            m["bias2"] = bias2
            m["biasv"] = np.tile(b_UV[heads][None, :], (128, 1)).astype(np.float32)
        in_maps.append(m)

    import os
    os.environ.setdefault("BASS_NEVER_TRACE", "1")
    res = run_bass_kernel_spmd(nc, in_maps, core_ids=list(range(8)))

    out = np.empty((B, T, DM), np.float32)
    for c in range(8):
        b, g = divmod(c, 4)
        ot = res.results[c]["out_t"]                    # [512, T]
        for h in range(HL):
            out[b, :, (4 * g + h) * DH: (4 * g + h + 1) * DH] = \
                ot[128 * h: 128 * (h + 1), :].T
    return out

